# revision 1
# baseline (speedup 1.0000x reference)
"""MoE dispatched linear (nn_DMoELinear) on 8 TRN2 NeuronCores.

out[t] = W[ids[t]] @ x[t] + b[ids[t]], computed in bf16 (matching the
reference, which casts x/W/b to bf16 before the grouped GEMM).

Strategy: expert parallelism. The host routes tokens by expert id
(all-to-all dispatch done on host since kernel() receives full inputs),
core e runs expert e's GEMM for its tokens at a shared static capacity
C = roundup(max_e count_e, 128), and the host scatters rows back.

Per-core GEMM (tokens on the PSUM partition dim):
    y[C, 2048] = xT[2048, C].T @ wT[2048, 2048] + bias   (bf16, f32 accum)
"""

import numpy as np
import ml_dtypes

E = 8          # experts == cores
IN_F = 2048
OUT_F = 2048
P = 128

_compile_cache = {}


def _build_nc(C):
    """Build + compile the per-core Bass program for token capacity C."""
    import concourse.mybir as mybir
    from concourse import bacc, tile
    from concourse.kernels.tile_matmul import matmul_tile_kernel

    nc = bacc.Bacc("TRN2", target_bir_lowering=False, debug=False)
    xT = nc.dram_tensor("xT", [IN_F, C], mybir.dt.bfloat16, kind="ExternalInput")
    wT = nc.dram_tensor("wT", [IN_F, OUT_F], mybir.dt.bfloat16, kind="ExternalInput")
    bias = nc.dram_tensor("bias", [P, OUT_F], mybir.dt.bfloat16, kind="ExternalInput")
    y = nc.dram_tensor("y", [C, OUT_F], mybir.dt.bfloat16, kind="ExternalOutput")

    with tile.TileContext(nc) as tc:
        with tc.tile_pool(name="const", bufs=1) as const:
            bias_sb = const.tile([P, OUT_F], mybir.dt.bfloat16)
            nc.sync.dma_start(bias_sb[:], bias[:])

            def add_bias(nc_, sbuf, md, _data):
                # sbuf: [128, m_subtiles, n_slice]; bias constant across
                # tokens (partitions+subtiles), varies along out features.
                start = md.n_tile_idx * md.n_tile
                b = bias_sb[:, None, start : start + sbuf.shape[2]]
                nc_.vector.tensor_add(
                    out=sbuf[:], in0=sbuf[:], in1=b.to_broadcast(sbuf.shape)
                )

            matmul_tile_kernel(
                tc,
                xT[:],
                wT[:],
                y[:],
                post_mxn_tile_fn=add_bias,
            )
    nc.compile()
    return nc


def _route(x, ids):
    """Host-side dispatch: group token indices by expert."""
    ids_flat = np.asarray(ids).reshape(-1).astype(np.int64)
    order = np.argsort(ids_flat, kind="stable")
    counts = np.bincount(ids_flat, minlength=E)
    C = max(int(counts.max()), P)
    C = -(-C // P) * P  # round up to multiple of 128
    starts = np.zeros(E + 1, np.int64)
    np.cumsum(counts, out=starts[1:])
    return order, counts, starts, C


def kernel(x, ids, weight, bias):
    from concourse.bass_utils import run_bass_kernel_spmd

    x = np.asarray(x)
    ids = np.asarray(ids)
    weight = np.asarray(weight)
    bias = np.asarray(bias)
    out_shape = (*x.shape[:-1], weight.shape[1])

    x_flat = x.reshape(-1, x.shape[-1])
    order, counts, starts, C = _route(x, ids)

    bf16 = ml_dtypes.bfloat16
    w_bf = weight.astype(bf16)
    b_bf = bias.astype(bf16)

    in_maps = []
    for e in range(E):
        idx = order[starts[e] : starts[e + 1]]
        xT_e = np.zeros((IN_F, C), dtype=bf16)
        xT_e[:, : counts[e]] = np.ascontiguousarray(x_flat[idx].astype(bf16).T)
        wT_e = np.ascontiguousarray(w_bf[e].T)
        bias_e = np.broadcast_to(b_bf[e], (P, OUT_F)).copy()
        in_maps.append({"xT": xT_e, "wT": wT_e, "bias": bias_e})

    if C not in _compile_cache:
        _compile_cache[C] = _build_nc(C)
    nc = _compile_cache[C]

    res = run_bass_kernel_spmd(nc, in_maps, core_ids=list(range(E)))

    out_flat = np.zeros((x_flat.shape[0], OUT_F), dtype=bf16)
    for e in range(E):
        idx = order[starts[e] : starts[e + 1]]
        out_flat[idx] = res.results[e]["y"][: counts[e]]
    return out_flat.reshape(out_shape)


# Exposed for test.py: run with tracing and return (out, BassKernelResults).
def _run_traced(x, ids, weight, bias, tmpdir=None):
    from concourse.bass_utils import run_bass_kernel_spmd

    x = np.asarray(x)
    weight = np.asarray(weight)
    bias = np.asarray(bias)
    out_shape = (*x.shape[:-1], weight.shape[1])
    x_flat = x.reshape(-1, x.shape[-1])
    order, counts, starts, C = _route(x, ids)

    bf16 = ml_dtypes.bfloat16
    w_bf = weight.astype(bf16)
    b_bf = bias.astype(bf16)
    in_maps = []
    for e in range(E):
        idx = order[starts[e] : starts[e + 1]]
        xT_e = np.zeros((IN_F, C), dtype=bf16)
        xT_e[:, : counts[e]] = np.ascontiguousarray(x_flat[idx].astype(bf16).T)
        wT_e = np.ascontiguousarray(w_bf[e].T)
        bias_e = np.broadcast_to(b_bf[e], (P, OUT_F)).copy()
        in_maps.append({"xT": xT_e, "wT": wT_e, "bias": bias_e})

    if C not in _compile_cache:
        _compile_cache[C] = _build_nc(C)
    nc = _compile_cache[C]

    res = run_bass_kernel_spmd(
        nc, in_maps, core_ids=list(range(E)), trace=True, tmpdir=tmpdir
    )
    out_flat = np.zeros((x_flat.shape[0], OUT_F), dtype=bf16)
    for e in range(E):
        idx = order[starts[e] : starts[e + 1]]
        out_flat[idx] = res.results[e]["y"][: counts[e]]
    return out_flat.reshape(out_shape), res
